# revision 1
# baseline (speedup 1.0000x reference)
"""Trainium2 Bass kernel for CheemsNonWoAttention (GQA attention, no output proj).

Sharding: 16 q-heads across 8 cores (2 q-heads + their shared kv-head per
core), SPMD with no collectives.  Each core computes its slice of the output
hidden dim; the host concatenates.

Math notes:
  - The reference's logn scale is max(log(65..80)/log(256), 1) == 1.0 -> no-op.
  - 1/sqrt(HD) score scale is folded into Wq on the host.
  - Softmax runs without max-subtraction (scores ~ N(0,1) + additive mask;
    exp underflows to 0 for very negative masks, which is exactly right).
    Scores are computed transposed, sT[k, q], so exp(sT) feeds attn@V
    directly as the moving operand (V chunks stationary), denominators come
    from a ones-vector matmul, and only the final [d, q] -> [q, d] flip
    needs PE transposes.
  - Matmuls run in float32r (TF32-like, ~1.5e-4 rms rel err per dot;
    ~3e-4 rms end-to-end).  fp32r moving dims must be even and >= 256 for
    full rate; weight loads are the per-matmul bottleneck, so walrus's
    ldw-opt pass is enabled via _patch_ldw_opt (~13% end-to-end).
  - The host inspects the mask and dispatches one of three compiled
    variants: "causal" (skips fully-masked k-chunks and the mask add on
    fully-unmasked ones), "zeros" (no mask work at all), "general"
    (arbitrary additive mask).
"""

import sys

if "/opt/trn_rl_repo" not in sys.path:
    sys.path.insert(0, "/opt/trn_rl_repo")

import math
import numpy as np

B, S, HID = 2, 2048, 2048
NH, NKV, HD = 16, 4, 128
NCORES = 8
HPC = NH // NCORES          # q heads per core
FPC = HPC * HD              # output features per core
KVW = HD                    # kv head width per core
P = 128
NCH = HID // P              # hid chunks (contraction tiles)
TT = 512                    # token tile, phase 1
QT = 512                    # q tile, phase 2
NKC = S // P                # k chunks

_CACHE = {}


def _patch_ldw_opt():
    # walrus's LDWEIGHTS dedup/overlap pass is off by default in the driver
    # args; it is worth ~13% end-to-end here (weight loads dominate fp32r
    # matmul issue otherwise).  Results verified identical with it on.
    import concourse.bass_utils as bu

    if getattr(bu, "_ldw_opt_patched", False):
        return
    orig = bu.run_command

    def patched(argv, **kw):
        argv = ["--enable-ldw-opt=true" if a == "--enable-ldw-opt=false" else a
                for a in argv]
        return orig(argv, **kw)

    bu.run_command = patched
    bu._ldw_opt_patched = True


def _build_nc(variant):
    _patch_ldw_opt()
    import concourse.bacc as bacc
    from concourse import mybir
    from concourse.tile import TileContext

    f32 = mybir.dt.float32
    f32r = mybir.dt.float32r
    bf16 = mybir.dt.bfloat16
    Exp = mybir.ActivationFunctionType.Exp

    nc = bacc.Bacc("TRN2", target_bir_lowering=False, debug=False, num_devices=NCORES)
    xT = nc.dram_tensor("xT", [B, HID, S], f32r, kind="ExternalInput").ap()
    wq = nc.dram_tensor("wq", [HID, FPC], f32r, kind="ExternalInput").ap()
    wk = nc.dram_tensor("wk", [HID, KVW], f32r, kind="ExternalInput").ap()
    wv = nc.dram_tensor("wv", [HID, KVW], f32r, kind="ExternalInput").ap()
    maskT = nc.dram_tensor("maskT", [B, S, S], bf16, kind="ExternalInput").ap()
    ident_d = nc.dram_tensor("ident", [P, P], f32r, kind="ExternalInput").ap()
    ident32_d = nc.dram_tensor("ident32", [P, P], f32, kind="ExternalInput").ap()
    ones_d = nc.dram_tensor("ones", [P, 1], f32r, kind="ExternalInput").ap()
    out = nc.dram_tensor("out", [B, S, FPC], f32, kind="ExternalOutput").ap()

    def active_kchunks(q0):
        if variant == "causal":
            return list(range(q0 // P + QT // P))
        return list(range(NKC))

    def masked_kchunks(q0):
        if variant == "causal":
            return set(range(q0 // P, q0 // P + QT // P))
        if variant == "zeros":
            return set()
        return set(range(NKC))

    with TileContext(nc) as tc:
        with tc.tile_pool(name="persist", bufs=1) as persist:
            wq_sb = persist.tile([P, NCH, FPC], f32r, tag="wq")
            wk_sb = persist.tile([P, NCH, KVW], f32r, tag="wk")
            wv_sb = persist.tile([P, NCH, KVW], f32r, tag="wv")
            ident = persist.tile([P, P], f32r, tag="ident")
            ident32 = persist.tile([P, P], f32, tag="ident32")
            ones_sb = persist.tile([P, 1], f32r, tag="ones")
            qT_sb = [persist.tile([P, HPC, S], f32r, tag=f"qT{b}", name=f"qT{b}") for b in range(B)]
            kT_sb = [persist.tile([P, S], f32r, tag=f"kT{b}", name=f"kT{b}") for b in range(B)]
            v_sb = [persist.tile([P, S], f32r, tag=f"v{b}", name=f"v{b}") for b in range(B)]

            nc.sync.dma_start(out=wq_sb[:], in_=wq.rearrange("(c p) f -> p c f", p=P))
            nc.gpsimd.dma_start(out=wk_sb[:], in_=wk.rearrange("(c p) f -> p c f", p=P))
            nc.gpsimd.dma_start(out=wv_sb[:], in_=wv.rearrange("(c p) f -> p c f", p=P))
            nc.gpsimd.dma_start(out=ident[:], in_=ident_d[:])
            nc.gpsimd.dma_start(out=ident32[:], in_=ident32_d[:])
            nc.gpsimd.dma_start(out=ones_sb[:], in_=ones_d[:])

            # ---------------- phase 1: Q/K/V projections ----------------
            with tc.tile_pool(name="xt", bufs=2) as xpool, \
                 tc.tile_pool(name="vst", bufs=2) as vstage, \
                 tc.tile_pool(name="ppsum", bufs=4, space="PSUM") as ppsum, \
                 tc.tile_pool(name="tpsum", bufs=2, space="PSUM") as tpsum:
                XSUB = 4                      # hid chunks per xt sub-tile
                NSUB = NCH // XSUB
                for b in range(B):
                    for t0 in range(0, S, TT):
                        xts = []
                        for s in range(NSUB):
                            xs = xpool.tile([P, XSUB, TT], f32r, tag=f"xt{s}",
                                            name=f"xt{s}_{b}_{t0}")
                            nc.sync.dma_start(
                                out=xs[:],
                                in_=xT[b, s * XSUB * P:(s + 1) * XSUB * P, t0:t0 + TT]
                                .rearrange("(c p) t -> p c t", p=P),
                            )
                            xts.append(xs)
                        xt = None
                        for h in range(HPC):
                            ps = ppsum.tile([P, TT], f32, tag="pp")
                            for c in range(NCH):
                                nc.tensor.matmul(
                                    ps[:],
                                    lhsT=wq_sb[:, c, h * HD:(h + 1) * HD],
                                    rhs=xts[c // XSUB][:, c % XSUB, :],
                                    start=(c == 0), stop=(c == NCH - 1),
                                )
                            nc.scalar.mul(out=qT_sb[b][:, h, t0:t0 + TT], in_=ps[:], mul=1.0)
                        ps = ppsum.tile([P, TT], f32, tag="pp")
                        for c in range(NCH):
                            nc.tensor.matmul(
                                ps[:], lhsT=wk_sb[:, c, :], rhs=xts[c // XSUB][:, c % XSUB, :],
                                start=(c == 0), stop=(c == NCH - 1),
                            )
                        nc.scalar.mul(out=kT_sb[b][:, t0:t0 + TT], in_=ps[:], mul=1.0)
                        ps = ppsum.tile([P, TT], f32, tag="pp")
                        for c in range(NCH):
                            nc.tensor.matmul(
                                ps[:], lhsT=wv_sb[:, c, :], rhs=xts[c // XSUB][:, c % XSUB, :],
                                start=(c == 0), stop=(c == NCH - 1),
                            )
                        vt = vstage.tile([P, TT], f32r, tag="vt")
                        nc.vector.tensor_copy(vt[:], ps[:])
                        for j in range(TT // P):
                            tp = tpsum.tile([P, P], f32r, tag="tp")
                            nc.tensor.transpose(tp[:], vt[:, j * P:(j + 1) * P], ident[:])
                            kc = t0 // P + j
                            nc.vector.tensor_copy(v_sb[b][:, kc * HD: (kc + 1) * HD], tp[:])

            # ---------------- phase 2+3: attention ----------------
            with tc.tile_pool(name="mask", bufs=4) as mpool, \
                 tc.tile_pool(name="et", bufs=1) as epool, \
                 tc.tile_pool(name="ot", bufs=2) as otpool, \
                 tc.tile_pool(name="small", bufs=8) as small, \
                 tc.tile_pool(name="spsum", bufs=3, space="PSUM") as spsum, \
                 tc.tile_pool(name="opsum", bufs=2, space="PSUM") as opsum, \
                 tc.tile_pool(name="supsum", bufs=1, space="PSUM") as supsum, \
                 tc.tile_pool(name="tpsum2", bufs=1, space="PSUM") as tpsum2:
                for b in range(B):
                    for q0 in range(0, S, QT):
                        act = active_kchunks(q0)
                        msk = masked_kchunks(q0)
                        et = [epool.tile([P, NKC, QT], f32r, tag=f"et{h}", name=f"et{h}_{b}_{q0}")
                              for h in range(HPC)]
                        # scores + exp, per k-chunk (mask tile shared by both heads)
                        for kc in act:
                            if kc in msk:
                                mt = mpool.tile([P, QT], bf16, tag="mt")
                                nc.sync.dma_start(
                                    out=mt[:], in_=maskT[b, kc * P:(kc + 1) * P, q0:q0 + QT]
                                )
                            for h in range(HPC):
                                sp = spsum.tile([P, QT], f32, tag="sp")
                                nc.tensor.matmul(
                                    sp[:],
                                    lhsT=kT_sb[b][:, kc * P:(kc + 1) * P],
                                    rhs=qT_sb[b][:, h, q0:q0 + QT],
                                    start=True, stop=True,
                                )
                                if kc in msk:
                                    nc.vector.tensor_add(out=sp[:], in0=sp[:], in1=mt[:])
                                nc.scalar.activation(out=et[h][:, kc, :], in_=sp[:], func=Exp)
                        # attn @ V (out^T form)
                        po = {}
                        for h in range(HPC):
                            po[h] = opsum.tile([P, QT], f32, tag="po", name=f"po{h}_{b}_{q0}")
                            for kc in act:
                                nc.tensor.matmul(
                                    po[h][:],
                                    lhsT=v_sb[b][:, kc * HD:(kc + 1) * HD],
                                    rhs=et[h][:, kc, :],
                                    start=(kc == act[0]), stop=(kc == act[-1]),
                                )
                        srow2 = small.tile([32 * (HPC - 1) + 1, QT], f32, tag="srow2")
                        oT = {}
                        for h in range(HPC):
                            psums = supsum.tile([P, QT], f32, tag="ps_sums")
                            for kc in act:
                                nc.tensor.matmul(
                                    psums[:1, :],
                                    lhsT=ones_sb[:, :1],
                                    rhs=et[h][:, kc, :],
                                    start=(kc == act[0]), stop=(kc == act[-1]),
                                )
                            oT[h] = otpool.tile([P, QT], f32, tag=f"oT{h}", name=f"oT{h}_{b}_{q0}")
                            nc.scalar.mul(out=oT[h][:], in_=po[h][:], mul=1.0)
                            nc.scalar.mul(out=srow2[32 * h:32 * h + 1, :], in_=psums[:1, :], mul=1.0)
                        # flip [d, q] -> [q, d] and normalize per-q
                        for qj in range(QT // P):
                            tps = tpsum2.tile([P, P], f32, tag="tps")
                            nw = 32 * (HPC - 1) + 1
                            nc.tensor.transpose(
                                tps[:, :nw], srow2[:, qj * P:(qj + 1) * P],
                                ident32[:nw, :nw],
                            )
                            rc = small.tile([P, HPC], f32, tag="rc")
                            for h in range(HPC):
                                nc.vector.reciprocal(rc[:, h:h + 1], tps[:, 32 * h:32 * h + 1])
                            for h in range(HPC):
                                tpo = tpsum2.tile([P, P], f32, tag="tpo")
                                nc.tensor.transpose(
                                    tpo[:], oT[h][:, qj * P:(qj + 1) * P], ident32[:]
                                )
                                ob = small.tile([P, HD], f32, tag="ob")
                                nc.vector.tensor_scalar_mul(ob[:], tpo[:], rc[:, h:h + 1])
                                nc.sync.dma_start(
                                    out=out[b, q0 + qj * P: q0 + (qj + 1) * P, h * HD:(h + 1) * HD],
                                    in_=ob[:],
                                )

    nc.compile()
    return nc


def get_nc(variant="general"):
    if variant not in _CACHE:
        _CACHE[variant] = _build_nc(variant)
    return _CACHE[variant]


def detect_variant(attention_mask):
    m = np.asarray(attention_mask, dtype=np.float32)[:, 0]   # [B, S, S] (q, k)
    if not np.any(m):
        return "zeros"
    # causal: zero on/below the diagonal, <= -1e8 strictly above
    kk = np.arange(S)
    lower = kk[None, :] <= kk[:, None]                       # [S(q), S(k)]
    for b in range(m.shape[0]):
        if np.any(m[b][lower] != 0.0):
            return "general"
        if np.any(m[b][~lower] > -1e8):
            return "general"
    return "causal"


def make_in_maps(hidden_states, attention_mask, Wq, Wk, Wv):
    import ml_dtypes

    xT = np.ascontiguousarray(
        np.asarray(hidden_states, dtype=np.float32).transpose(0, 2, 1)
    )
    mT = np.ascontiguousarray(
        np.asarray(attention_mask, dtype=np.float32)[:, 0].transpose(0, 2, 1)
    ).astype(ml_dtypes.bfloat16)
    wq_s = (np.asarray(Wq, dtype=np.float32) / math.sqrt(HD)).astype(np.float32)
    wk = np.asarray(Wk, dtype=np.float32)
    wv = np.asarray(Wv, dtype=np.float32)
    ident = np.eye(P, dtype=np.float32)
    ones = np.ones((P, 1), dtype=np.float32)

    in_maps = []
    for c in range(NCORES):
        kv = c // 2
        in_maps.append({
            "xT": xT,
            "wq": np.ascontiguousarray(wq_s[:, c * FPC:(c + 1) * FPC]),
            "wk": np.ascontiguousarray(wk[:, kv * KVW:(kv + 1) * KVW]),
            "wv": np.ascontiguousarray(wv[:, kv * KVW:(kv + 1) * KVW]),
            "maskT": mT,
            "ident": ident,
            "ident32": ident,
            "ones": ones,
        })
    return in_maps


def kernel(hidden_states, attention_mask, Wq, Wk, Wv):
    from concourse.bass_utils import run_bass_kernel_spmd

    variant = detect_variant(attention_mask)
    nc = get_nc(variant)
    in_maps = make_in_maps(hidden_states, attention_mask, Wq, Wk, Wv)
    res = run_bass_kernel_spmd(nc, in_maps, core_ids=list(range(NCORES)))
    outs = [res.results[c]["out"] for c in range(NCORES)]
    return np.concatenate(outs, axis=2).astype(np.float32)



# revision 4
# speedup vs baseline: 1.0943x; 1.0943x over previous
"""Trainium2 Bass kernel for CheemsNonWoAttention (GQA attention, no output proj).

Sharding: 16 q-heads across 8 cores (2 q-heads + their shared kv-head per
core), SPMD with no collectives.  Each core computes its slice of the output
hidden dim; the host concatenates and transposes.

Math notes (v2 — PE-lean layout):
  - The reference's logn scale is max(log(65..80)/log(256), 1) == 1.0 -> no-op.
  - 1/sqrt(HD) score scale is folded into Wq on the host.
  - All matmul operands are bf16 (halves HBM traffic and SBUF; PE rate is
    identical to fp32r).  PSUM accumulation stays fp32.
  - Scores are computed transposed, sT[k, q]; exp(sT) feeds attn@V directly
    as the moving operand (V chunks stationary).
  - Softmax runs without max-subtraction (scores ~ N(0,1); exp of the causal
    -1e9 mask underflows to 0, which is exactly right).
  - Denominators come from a chain with an ALL-ONES [128,128] stationary:
    out[m, q] = sum_k exp[k, q] for every m, i.e. the row-sum replicated
    across all 128 partitions.  A [128,512] DVE reciprocal + tensor_mul then
    normalizes po in place -- NO transposes anywhere in the epilogue.  The
    output is written as out[b, d, q] and the host transposes.
  - Causal variant: fully-masked k-chunks are skipped; the diagonal chunk j
    of a q-block only computes the live columns [128j:512] (restricted
    moving operand), and the mask add collapses to a single shared
    [128,128] triangle constant applied to a 128-col window.
"""

import sys

if "/opt/trn_rl_repo" not in sys.path:
    sys.path.insert(0, "/opt/trn_rl_repo")

import math
import numpy as np

B, S, HID = 2, 2048, 2048
NH, NKV, HD = 16, 4, 128
NCORES = 8
HPC = NH // NCORES          # q heads per core
FPC = HPC * HD              # output features per core
KVW = HD                    # kv head width per core
P = 128
NCH = HID // P              # hid chunks (contraction tiles)
TT = 512                    # token tile, phase 1
QT = 512                    # q tile, phase 2
NKC = S // P                # k chunks

_CACHE = {}


def _patch_ldw_opt():
    # walrus's LDWEIGHTS dedup/overlap pass is off by default in the driver
    # args; weight loads otherwise crowd the PE issue stream.
    import concourse.bass_utils as bu

    if getattr(bu, "_ldw_opt_patched", False):
        return
    orig = bu.run_command

    def patched(argv, **kw):
        argv = ["--enable-ldw-opt=true" if a == "--enable-ldw-opt=false" else a
                for a in argv]
        return orig(argv, **kw)

    bu.run_command = patched
    bu._ldw_opt_patched = True


def _build_nc(variant):
    # NOTE: the fp32r-era ldw-opt patch is OFF: walrus's LDW-opt pass
    # rejects the bf16-emitted InstLdweights ("not compatible with LDW
    # optimization").  bf16 weights get FWL automatically, so explicit
    # weight loads are cheap without it.
    import concourse.bacc as bacc
    from concourse import mybir
    from concourse.tile import TileContext

    f32 = mybir.dt.float32
    bf16 = mybir.dt.bfloat16
    Exp = mybir.ActivationFunctionType.Exp

    nc = bacc.Bacc("TRN2", target_bir_lowering=False, debug=False, num_devices=NCORES)
    xT = nc.dram_tensor("xT", [B, HID, S], bf16, kind="ExternalInput").ap()
    wq = nc.dram_tensor("wq", [HID, FPC], bf16, kind="ExternalInput").ap()
    wk = nc.dram_tensor("wk", [HID, KVW], bf16, kind="ExternalInput").ap()
    wv = nc.dram_tensor("wv", [HID, KVW], bf16, kind="ExternalInput").ap()
    ident_d = nc.dram_tensor("ident", [P, P], bf16, kind="ExternalInput").ap()
    ones_d = nc.dram_tensor("ones", [P, P], bf16, kind="ExternalInput").ap()
    if variant == "causal":
        tri_d = nc.dram_tensor("tri", [P, P], bf16, kind="ExternalInput").ap()
    if variant == "general":
        maskT = nc.dram_tensor("maskT", [B, S, S], bf16, kind="ExternalInput").ap()
    out = nc.dram_tensor("out", [B, FPC, S], f32, kind="ExternalOutput").ap()

    def chunks(q0):
        # [(kc, live_lo)] — live_lo is the first live column within the
        # q-block for that k-chunk (0 = fully live).
        if variant == "causal":
            full = [(kc, 0) for kc in range(q0 // P)]
            diag = [(q0 // P + j, j * P) for j in range(QT // P)]
            return full + diag
        return [(kc, 0) for kc in range(NKC)]

    def is_diag(kc, q0):
        return variant == "causal" and kc >= q0 // P

    with TileContext(nc) as tc:
        with tc.tile_pool(name="persist", bufs=1) as persist:
            wq_sb = persist.tile([P, NCH, FPC], bf16, tag="wq")
            wk_sb = persist.tile([P, NCH, KVW], bf16, tag="wk")
            wv_sb = persist.tile([P, NCH, KVW], bf16, tag="wv")
            ident = persist.tile([P, P], bf16, tag="ident")
            ones_sb = persist.tile([P, P], bf16, tag="ones")
            if variant == "causal":
                tri_sb = persist.tile([P, P], bf16, tag="tri")
            qT_sb = [persist.tile([P, HPC, S], bf16, tag=f"qT{b}", name=f"qT{b}") for b in range(B)]
            kT_sb = [persist.tile([P, S], bf16, tag=f"kT{b}", name=f"kT{b}") for b in range(B)]
            v_sb = [persist.tile([P, S], bf16, tag=f"v{b}", name=f"v{b}") for b in range(B)]

            nc.gpsimd.dma_start(out=ident[:], in_=ident_d[:])
            nc.gpsimd.dma_start(out=ones_sb[:], in_=ones_d[:])
            if variant == "causal":
                nc.gpsimd.dma_start(out=tri_sb[:], in_=tri_d[:])
            nc.sync.dma_start(out=wq_sb[:], in_=wq.rearrange("(c p) f -> p c f", p=P))
            nc.gpsimd.dma_start(out=wk_sb[:], in_=wk.rearrange("(c p) f -> p c f", p=P))
            nc.gpsimd.dma_start(out=wv_sb[:], in_=wv.rearrange("(c p) f -> p c f", p=P))

            # ---------------- phase 1: Q/K/V projections ----------------
            with tc.tile_pool(name="xt", bufs=2) as xpool, \
                 tc.tile_pool(name="vst", bufs=2) as vstage, \
                 tc.tile_pool(name="warm", bufs=1, space="PSUM") as wpsum, \
                 tc.tile_pool(name="ppsum", bufs=3, space="PSUM") as ppsum, \
                 tc.tile_pool(name="tpsum", bufs=2, space="PSUM") as tpsum:
                # HAM warmup: harmless matmuls on the identity while the
                # first xT tile's DMA is in flight.
                wp = wpsum.tile([P, P], f32, tag="warm")
                for _ in range(24):
                    nc.tensor.matmul(wp[:], lhsT=ident[:], rhs=ident[:],
                                     start=True, stop=True)
                XSUB = 4                      # hid chunks per xt sub-tile
                NSUB = NCH // XSUB
                for b in range(B):
                    for t0 in range(0, S, TT):
                        xts = []
                        for s in range(NSUB):
                            xs = xpool.tile([P, XSUB, TT], bf16, tag=f"xt{s}",
                                            name=f"xt{s}_{b}_{t0}")
                            nc.sync.dma_start(
                                out=xs[:],
                                in_=xT[b, s * XSUB * P:(s + 1) * XSUB * P, t0:t0 + TT]
                                .rearrange("(c p) t -> p c t", p=P),
                            )
                            xts.append(xs)
                        for h in range(HPC):
                            ps = ppsum.tile([P, TT], f32, tag="pp")
                            for c in range(NCH):
                                nc.tensor.matmul(
                                    ps[:],
                                    lhsT=wq_sb[:, c, h * HD:(h + 1) * HD],
                                    rhs=xts[c // XSUB][:, c % XSUB, :],
                                    start=(c == 0), stop=(c == NCH - 1),
                                )
                            nc.scalar.mul(out=qT_sb[b][:, h, t0:t0 + TT], in_=ps[:], mul=1.0)
                        ps = ppsum.tile([P, TT], f32, tag="pp")
                        for c in range(NCH):
                            nc.tensor.matmul(
                                ps[:], lhsT=wk_sb[:, c, :], rhs=xts[c // XSUB][:, c % XSUB, :],
                                start=(c == 0), stop=(c == NCH - 1),
                            )
                        nc.scalar.mul(out=kT_sb[b][:, t0:t0 + TT], in_=ps[:], mul=1.0)
                        ps = ppsum.tile([P, TT], f32, tag="pp")
                        for c in range(NCH):
                            nc.tensor.matmul(
                                ps[:], lhsT=wv_sb[:, c, :], rhs=xts[c // XSUB][:, c % XSUB, :],
                                start=(c == 0), stop=(c == NCH - 1),
                            )
                        vt = vstage.tile([P, TT], bf16, tag="vt")
                        nc.vector.tensor_copy(vt[:], ps[:])
                        for j in range(TT // P):
                            tp = tpsum.tile([P, P], bf16, tag="tp")
                            nc.tensor.transpose(tp[:], vt[:, j * P:(j + 1) * P], ident[:])
                            kc = t0 // P + j
                            nc.vector.tensor_copy(v_sb[b][:, kc * HD: (kc + 1) * HD], tp[:])

            # ---------------- phase 2+3: attention ----------------
            with tc.tile_pool(name="mask", bufs=4) as mpool, \
                 tc.tile_pool(name="et", bufs=2) as epool, \
                 tc.tile_pool(name="rc", bufs=4) as rcpool, \
                 tc.tile_pool(name="ob", bufs=4) as obpool, \
                 tc.tile_pool(name="spsum", bufs=3, space="PSUM") as spsum, \
                 tc.tile_pool(name="opsum", bufs=2, space="PSUM") as opsum, \
                 tc.tile_pool(name="supsum", bufs=2, space="PSUM") as supsum:
                for b in range(B):
                    for q0 in range(0, S, QT):
                        act = chunks(q0)
                        et = [epool.tile([P, NKC, QT], bf16, tag=f"et{h}", name=f"et{h}_{b}_{q0}")
                              for h in range(HPC)]
                        # scores + exp, per k-chunk
                        for kc, lo in act:
                            if variant == "general":
                                mt = mpool.tile([P, QT], bf16, tag="mt")
                                nc.sync.dma_start(
                                    out=mt[:], in_=maskT[b, kc * P:(kc + 1) * P, q0:q0 + QT]
                                )
                            for h in range(HPC):
                                sp = spsum.tile([P, QT], f32, tag="sp")
                                nc.tensor.matmul(
                                    sp[:, lo:],
                                    lhsT=kT_sb[b][:, kc * P:(kc + 1) * P],
                                    rhs=qT_sb[b][:, h, q0 + lo:q0 + QT],
                                    start=True, stop=True,
                                )
                                if is_diag(kc, q0):
                                    nc.vector.tensor_add(
                                        out=sp[:, lo:lo + P], in0=sp[:, lo:lo + P],
                                        in1=tri_sb[:],
                                    )
                                elif variant == "general":
                                    nc.vector.tensor_add(out=sp[:], in0=sp[:], in1=mt[:])
                                nc.scalar.activation(
                                    out=et[h][:, kc, lo:], in_=sp[:, lo:], func=Exp
                                )
                        # attn @ V (out^T form) + denominators
                        for h in range(HPC):
                            po = opsum.tile([P, QT], f32, tag="po", name=f"po{h}_{b}_{q0}")
                            for i, (kc, lo) in enumerate(act):
                                nc.tensor.matmul(
                                    po[:, lo:],
                                    lhsT=v_sb[b][:, kc * HD:(kc + 1) * HD],
                                    rhs=et[h][:, kc, lo:],
                                    start=(i == 0), stop=(i == len(act) - 1),
                                )
                            sm = supsum.tile([P, QT], f32, tag="sm", name=f"sm{h}_{b}_{q0}")
                            for i, (kc, lo) in enumerate(act):
                                nc.tensor.matmul(
                                    sm[:, lo:],
                                    lhsT=ones_sb[:],
                                    rhs=et[h][:, kc, lo:],
                                    start=(i == 0), stop=(i == len(act) - 1),
                                )
                            rc = rcpool.tile([P, QT], f32, tag="rc", name=f"rc{h}_{b}_{q0}")
                            nc.vector.reciprocal(rc[:], sm[:])
                            ob = obpool.tile([P, QT], f32, tag="ob", name=f"ob{h}_{b}_{q0}")
                            nc.vector.tensor_mul(ob[:], po[:], rc[:])
                            nc.sync.dma_start(
                                out=out[b, h * HD:(h + 1) * HD, q0:q0 + QT], in_=ob[:]
                            )

    nc.compile()
    return nc


def get_nc(variant="general"):
    if variant not in _CACHE:
        _CACHE[variant] = _build_nc(variant)
    return _CACHE[variant]


def detect_variant(attention_mask):
    m = np.asarray(attention_mask, dtype=np.float32)[:, 0]   # [B, S, S] (q, k)
    if not np.any(m):
        return "zeros"
    # causal: zero on/below the diagonal, <= -1e8 strictly above
    kk = np.arange(S)
    lower = kk[None, :] <= kk[:, None]                       # [S(q), S(k)]
    for b in range(m.shape[0]):
        if np.any(m[b][lower] != 0.0):
            return "general"
        if np.any(m[b][~lower] > -1e8):
            return "general"
    return "causal"


def make_in_maps(variant, hidden_states, attention_mask, Wq, Wk, Wv):
    import ml_dtypes

    bf = ml_dtypes.bfloat16
    xT = np.ascontiguousarray(
        np.asarray(hidden_states, dtype=np.float32).transpose(0, 2, 1)
    ).astype(bf)
    wq_s = (np.asarray(Wq, dtype=np.float32) / math.sqrt(HD)).astype(bf)
    wk = np.asarray(Wk, dtype=np.float32).astype(bf)
    wv = np.asarray(Wv, dtype=np.float32).astype(bf)
    ident = np.eye(P, dtype=np.float32).astype(bf)
    ones = np.ones((P, P), dtype=np.float32).astype(bf)
    rr, cc = np.arange(P)[:, None], np.arange(P)[None, :]
    tri = np.where(rr <= cc, 0.0, -1e9).astype(np.float32).astype(bf)

    in_maps = []
    for c in range(NCORES):
        kv = c // 2
        m = {
            "xT": xT,
            "wq": np.ascontiguousarray(wq_s[:, c * FPC:(c + 1) * FPC]),
            "wk": np.ascontiguousarray(wk[:, kv * KVW:(kv + 1) * KVW]),
            "wv": np.ascontiguousarray(wv[:, kv * KVW:(kv + 1) * KVW]),
            "ident": ident,
            "ones": ones,
        }
        if variant == "causal":
            m["tri"] = tri
        if variant == "general":
            m["maskT"] = np.ascontiguousarray(
                np.asarray(attention_mask, dtype=np.float32)[:, 0].transpose(0, 2, 1)
            ).astype(bf)
        in_maps.append(m)
    return in_maps


def kernel(hidden_states, attention_mask, Wq, Wk, Wv):
    from concourse.bass_utils import run_bass_kernel_spmd

    variant = detect_variant(attention_mask)
    nc = get_nc(variant)
    in_maps = make_in_maps(variant, hidden_states, attention_mask, Wq, Wk, Wv)
    res = run_bass_kernel_spmd(nc, in_maps, core_ids=list(range(NCORES)))
    full = np.concatenate([res.results[c]["out"] for c in range(NCORES)], axis=1)
    return np.ascontiguousarray(full.transpose(0, 2, 1)).astype(np.float32)


# revision 6
# speedup vs baseline: 1.3109x; 1.1979x over previous
"""Trainium2 Bass kernel for CheemsNonWoAttention (GQA attention, no output proj).

Sharding: 16 q-heads across 8 cores (2 q-heads + their shared kv-head per
core), SPMD with no collectives.  Each core computes its slice of the output
hidden dim; the host concatenates and transposes.

Math notes (v2 — PE-lean layout):
  - The reference's logn scale is max(log(65..80)/log(256), 1) == 1.0 -> no-op.
  - 1/sqrt(HD) score scale is folded into Wq on the host.
  - All matmul operands are bf16 (halves HBM traffic and SBUF; PE rate is
    identical to fp32r).  PSUM accumulation stays fp32.
  - Scores are computed transposed, sT[k, q]; exp(sT) feeds attn@V directly
    as the moving operand (V chunks stationary).
  - Softmax runs without max-subtraction (scores ~ N(0,1); exp of the causal
    -1e9 mask underflows to 0, which is exactly right).
  - Denominators come from a chain with an ALL-ONES [128,128] stationary:
    out[m, q] = sum_k exp[k, q] for every m, i.e. the row-sum replicated
    across all 128 partitions.  A [128,512] DVE reciprocal + tensor_mul then
    normalizes po in place -- NO transposes anywhere in the epilogue.  The
    output is written as out[b, d, q] and the host transposes.
  - Causal variant: fully-masked k-chunks are skipped; the diagonal chunk j
    of a q-block only computes the live columns [128j:512] (restricted
    moving operand), and the mask add collapses to a single shared
    [128,128] triangle constant applied to a 128-col window.
"""

import sys

if "/opt/trn_rl_repo" not in sys.path:
    sys.path.insert(0, "/opt/trn_rl_repo")

import math
import numpy as np

B, S, HID = 2, 2048, 2048
NH, NKV, HD = 16, 4, 128
NCORES = 8
HPC = NH // NCORES          # q heads per core
FPC = HPC * HD              # output features per core
KVW = HD                    # kv head width per core
P = 128
NCH = HID // P              # hid chunks (contraction tiles)
TT = 512                    # token tile, phase 1
QT = 512                    # q tile, phase 2
NKC = S // P                # k chunks

_CACHE = {}


def _patch_ldw_opt():
    # walrus's LDWEIGHTS dedup/overlap pass is off by default in the driver
    # args; weight loads otherwise crowd the PE issue stream.
    import concourse.bass_utils as bu

    if getattr(bu, "_ldw_opt_patched", False):
        return
    orig = bu.run_command

    def patched(argv, **kw):
        argv = ["--enable-ldw-opt=true" if a == "--enable-ldw-opt=false" else a
                for a in argv]
        return orig(argv, **kw)

    bu.run_command = patched
    bu._ldw_opt_patched = True


def _build_nc(variant):
    # NOTE: the fp32r-era ldw-opt patch is OFF: walrus's LDW-opt pass
    # rejects the bf16-emitted InstLdweights ("not compatible with LDW
    # optimization").  bf16 weights get FWL automatically, so explicit
    # weight loads are cheap without it.
    import concourse.bacc as bacc
    from concourse import mybir
    from concourse.tile import TileContext

    f32 = mybir.dt.float32
    bf16 = mybir.dt.bfloat16
    Exp = mybir.ActivationFunctionType.Exp

    nc = bacc.Bacc("TRN2", target_bir_lowering=False, debug=False, num_devices=NCORES)
    xT = nc.dram_tensor("xT", [B, HID, S], bf16, kind="ExternalInput").ap()
    wq = nc.dram_tensor("wq", [HID, FPC], bf16, kind="ExternalInput").ap()
    wk = nc.dram_tensor("wk", [HID, KVW], bf16, kind="ExternalInput").ap()
    wv = nc.dram_tensor("wv", [HID, KVW], bf16, kind="ExternalInput").ap()
    ident_d = nc.dram_tensor("ident", [P, P], bf16, kind="ExternalInput").ap()
    ones_d = nc.dram_tensor("ones", [P, P], bf16, kind="ExternalInput").ap()
    if variant == "causal":
        tri_d = nc.dram_tensor("tri", [P, P], bf16, kind="ExternalInput").ap()
    if variant == "general":
        maskT = nc.dram_tensor("maskT", [B, S, S], bf16, kind="ExternalInput").ap()
    out = nc.dram_tensor("out", [B, FPC, S], f32, kind="ExternalOutput").ap()

    def chunks(q0):
        # [(kc, live_lo)] — live_lo is the first live column within the
        # q-block for that k-chunk (0 = fully live).
        if variant == "causal":
            full = [(kc, 0) for kc in range(q0 // P)]
            diag = [(q0 // P + j, j * P) for j in range(QT // P)]
            return full + diag
        return [(kc, 0) for kc in range(NKC)]

    def is_diag(kc, q0):
        return variant == "causal" and kc >= q0 // P

    with TileContext(nc) as tc:
        with tc.tile_pool(name="persist", bufs=1) as persist:
            wq_sb = persist.tile([P, NCH, FPC], bf16, tag="wq")
            wk_sb = persist.tile([P, NCH, KVW], bf16, tag="wk")
            wv_sb = persist.tile([P, NCH, KVW], bf16, tag="wv")
            ident = persist.tile([P, P], bf16, tag="ident")
            ones_sb = persist.tile([P, P], bf16, tag="ones")
            if variant == "causal":
                tri_sb = persist.tile([P, P], bf16, tag="tri")
            qT_sb = [persist.tile([P, HPC, S], bf16, tag=f"qT{b}", name=f"qT{b}") for b in range(B)]
            kT_sb = [persist.tile([P, S], bf16, tag=f"kT{b}", name=f"kT{b}") for b in range(B)]
            v_sb = [persist.tile([P, S], bf16, tag=f"v{b}", name=f"v{b}") for b in range(B)]

            nc.gpsimd.dma_start(out=ident[:], in_=ident_d[:])
            nc.gpsimd.dma_start(out=ones_sb[:], in_=ones_d[:])
            if variant == "causal":
                nc.gpsimd.dma_start(out=tri_sb[:], in_=tri_d[:])
            nc.sync.dma_start(out=wq_sb[:], in_=wq.rearrange("(c p) f -> p c f", p=P))
            nc.gpsimd.dma_start(out=wk_sb[:], in_=wk.rearrange("(c p) f -> p c f", p=P))
            nc.gpsimd.dma_start(out=wv_sb[:], in_=wv.rearrange("(c p) f -> p c f", p=P))

            # ---------------- phase 1: Q/K/V projections ----------------
            with tc.tile_pool(name="xt", bufs=2) as xpool, \
                 tc.tile_pool(name="vst", bufs=2) as vstage, \
                 tc.tile_pool(name="warm", bufs=1, space="PSUM") as wpsum, \
                 tc.tile_pool(name="ppsum", bufs=3, space="PSUM") as ppsum, \
                 tc.tile_pool(name="tpsum", bufs=2, space="PSUM") as tpsum:
                # HAM warmup: harmless matmuls on the identity while the
                # first xT tile's DMA is in flight.
                wp = wpsum.tile([P, P], f32, tag="warm")
                for _ in range(24):
                    nc.tensor.matmul(wp[:], lhsT=ident[:], rhs=ident[:],
                                     start=True, stop=True)
                XSUB = 4                      # hid chunks per xt sub-tile
                NSUB = NCH // XSUB
                for b in range(B):
                    for t0 in range(0, S, TT):
                        xts = []
                        for s in range(NSUB):
                            xs = xpool.tile([P, XSUB, TT], bf16, tag=f"xt{s}",
                                            name=f"xt{s}_{b}_{t0}")
                            nc.sync.dma_start(
                                out=xs[:],
                                in_=xT[b, s * XSUB * P:(s + 1) * XSUB * P, t0:t0 + TT]
                                .rearrange("(c p) t -> p c t", p=P),
                            )
                            xts.append(xs)
                        for h in range(HPC):
                            ps = ppsum.tile([P, TT], f32, tag="pp")
                            for c in range(NCH):
                                nc.tensor.matmul(
                                    ps[:],
                                    lhsT=wq_sb[:, c, h * HD:(h + 1) * HD],
                                    rhs=xts[c // XSUB][:, c % XSUB, :],
                                    start=(c == 0), stop=(c == NCH - 1),
                                )
                            nc.scalar.mul(out=qT_sb[b][:, h, t0:t0 + TT], in_=ps[:], mul=1.0)
                        ps = ppsum.tile([P, TT], f32, tag="pp")
                        for c in range(NCH):
                            nc.tensor.matmul(
                                ps[:], lhsT=wk_sb[:, c, :], rhs=xts[c // XSUB][:, c % XSUB, :],
                                start=(c == 0), stop=(c == NCH - 1),
                            )
                        nc.scalar.mul(out=kT_sb[b][:, t0:t0 + TT], in_=ps[:], mul=1.0)
                        ps = ppsum.tile([P, TT], f32, tag="pp")
                        for c in range(NCH):
                            nc.tensor.matmul(
                                ps[:], lhsT=wv_sb[:, c, :], rhs=xts[c // XSUB][:, c % XSUB, :],
                                start=(c == 0), stop=(c == NCH - 1),
                            )
                        vt = vstage.tile([P, TT], bf16, tag="vt")
                        nc.vector.tensor_copy(vt[:], ps[:])
                        for j in range(TT // P):
                            tp = tpsum.tile([P, P], bf16, tag="tp")
                            nc.tensor.transpose(tp[:], vt[:, j * P:(j + 1) * P], ident[:])
                            kc = t0 // P + j
                            nc.vector.tensor_copy(v_sb[b][:, kc * HD: (kc + 1) * HD], tp[:])

            # ---------------- phase 2+3: attention ----------------
            with tc.tile_pool(name="mask", bufs=4) as mpool, \
                 tc.tile_pool(name="et", bufs=2) as epool, \
                 tc.tile_pool(name="rc", bufs=4) as rcpool, \
                 tc.tile_pool(name="ob", bufs=4) as obpool, \
                 tc.tile_pool(name="spsum", bufs=2, space="PSUM") as spsum, \
                 tc.tile_pool(name="opsum", bufs=2, space="PSUM") as opsum, \
                 tc.tile_pool(name="supsum", bufs=2, space="PSUM") as supsum:
                for b in range(B):
                    for q0 in range(0, S, QT):
                        act = chunks(q0)
                        et = epool.tile([P, NKC, HPC, QT], bf16, tag="et",
                                        name=f"et_{b}_{q0}")
                        # scores + exp, per k-chunk; both heads share one
                        # 2-bank psum tile so exp is a single ACT op.
                        for kc, lo in act:
                            if variant == "general":
                                mt = mpool.tile([P, QT], bf16, tag="mt")
                                nc.sync.dma_start(
                                    out=mt[:], in_=maskT[b, kc * P:(kc + 1) * P, q0:q0 + QT]
                                )
                            sp = spsum.tile([P, HPC, QT], f32, tag="sp")
                            for h in range(HPC):
                                nc.tensor.matmul(
                                    sp[:, h, lo:],
                                    lhsT=kT_sb[b][:, kc * P:(kc + 1) * P],
                                    rhs=qT_sb[b][:, h, q0 + lo:q0 + QT],
                                    start=True, stop=True,
                                )
                                if is_diag(kc, q0):
                                    nc.vector.tensor_add(
                                        out=sp[:, h, lo:lo + P], in0=sp[:, h, lo:lo + P],
                                        in1=tri_sb[:],
                                    )
                                elif variant == "general":
                                    nc.vector.tensor_add(
                                        out=sp[:, h, :], in0=sp[:, h, :], in1=mt[:]
                                    )
                            nc.scalar.activation(
                                out=et[:, kc, :, lo:], in_=sp[:, :, lo:], func=Exp
                            )
                        # denominators first (reciprocal overlaps attn@V),
                        # then attn @ V (out^T form), normalize, store.
                        for h in range(HPC):
                            sm = supsum.tile([P, QT], f32, tag="sm", name=f"sm{h}_{b}_{q0}")
                            for i, (kc, lo) in enumerate(act):
                                nc.tensor.matmul(
                                    sm[:, lo:],
                                    lhsT=ones_sb[:],
                                    rhs=et[:, kc, h, lo:],
                                    start=(i == 0), stop=(i == len(act) - 1),
                                )
                            rc = rcpool.tile([P, QT], f32, tag="rc", name=f"rc{h}_{b}_{q0}")
                            nc.vector.reciprocal_approx_fast(rc[:], sm[:])
                            po = opsum.tile([P, QT], f32, tag="po", name=f"po{h}_{b}_{q0}")
                            for i, (kc, lo) in enumerate(act):
                                nc.tensor.matmul(
                                    po[:, lo:],
                                    lhsT=v_sb[b][:, kc * HD:(kc + 1) * HD],
                                    rhs=et[:, kc, h, lo:],
                                    start=(i == 0), stop=(i == len(act) - 1),
                                )
                            ob = obpool.tile([P, QT], f32, tag="ob", name=f"ob{h}_{b}_{q0}")
                            nc.vector.tensor_mul(ob[:], po[:], rc[:])
                            nc.sync.dma_start(
                                out=out[b, h * HD:(h + 1) * HD, q0:q0 + QT], in_=ob[:]
                            )

    nc.compile()
    return nc


def get_nc(variant="general"):
    if variant not in _CACHE:
        _CACHE[variant] = _build_nc(variant)
    return _CACHE[variant]


def detect_variant(attention_mask):
    m = np.asarray(attention_mask, dtype=np.float32)[:, 0]   # [B, S, S] (q, k)
    if not np.any(m):
        return "zeros"
    # causal: zero on/below the diagonal, <= -1e8 strictly above
    kk = np.arange(S)
    lower = kk[None, :] <= kk[:, None]                       # [S(q), S(k)]
    for b in range(m.shape[0]):
        if np.any(m[b][lower] != 0.0):
            return "general"
        if np.any(m[b][~lower] > -1e8):
            return "general"
    return "causal"


def make_in_maps(variant, hidden_states, attention_mask, Wq, Wk, Wv):
    import ml_dtypes

    bf = ml_dtypes.bfloat16
    xT = np.ascontiguousarray(
        np.asarray(hidden_states, dtype=np.float32).transpose(0, 2, 1)
    ).astype(bf)
    wq_s = (np.asarray(Wq, dtype=np.float32) / math.sqrt(HD)).astype(bf)
    wk = np.asarray(Wk, dtype=np.float32).astype(bf)
    wv = np.asarray(Wv, dtype=np.float32).astype(bf)
    ident = np.eye(P, dtype=np.float32).astype(bf)
    ones = np.ones((P, P), dtype=np.float32).astype(bf)
    rr, cc = np.arange(P)[:, None], np.arange(P)[None, :]
    tri = np.where(rr <= cc, 0.0, -1e9).astype(np.float32).astype(bf)

    in_maps = []
    for c in range(NCORES):
        kv = c // 2
        m = {
            "xT": xT,
            "wq": np.ascontiguousarray(wq_s[:, c * FPC:(c + 1) * FPC]),
            "wk": np.ascontiguousarray(wk[:, kv * KVW:(kv + 1) * KVW]),
            "wv": np.ascontiguousarray(wv[:, kv * KVW:(kv + 1) * KVW]),
            "ident": ident,
            "ones": ones,
        }
        if variant == "causal":
            m["tri"] = tri
        if variant == "general":
            m["maskT"] = np.ascontiguousarray(
                np.asarray(attention_mask, dtype=np.float32)[:, 0].transpose(0, 2, 1)
            ).astype(bf)
        in_maps.append(m)
    return in_maps


def kernel(hidden_states, attention_mask, Wq, Wk, Wv):
    from concourse.bass_utils import run_bass_kernel_spmd

    variant = detect_variant(attention_mask)
    nc = get_nc(variant)
    in_maps = make_in_maps(variant, hidden_states, attention_mask, Wq, Wk, Wv)
    res = run_bass_kernel_spmd(nc, in_maps, core_ids=list(range(NCORES)))
    full = np.concatenate([res.results[c]["out"] for c in range(NCORES)], axis=1)
    return np.ascontiguousarray(full.transpose(0, 2, 1)).astype(np.float32)


# revision 7
# speedup vs baseline: 1.3115x; 1.0005x over previous
"""Trainium2 Bass kernel for CheemsNonWoAttention (GQA attention, no output proj).

Sharding (v4): core c handles batch c//4 and kv-head j=c%4 with its 4 q-heads
4j..4j+3.  KV work is perfectly sharded (no duplication), each core reads only
its batch's activations, SPMD with no collectives.  Each core writes its
[512, S] slice of out^T; the host concatenates and transposes.

Math notes:
  - The reference's logn scale is max(log(65..80)/log(256), 1) == 1.0 -> no-op.
  - 1/sqrt(HD) score scale is folded into Wq on the host.
  - All matmul operands are bf16 (halves HBM traffic and SBUF; PE rate is
    identical to fp32r).  PSUM accumulation stays fp32.
  - Scores are computed transposed, sT[k, q]; exp(sT) feeds attn@V directly
    as the moving operand (V chunks stationary).  Head pairs share one
    2-bank psum tile so exp is a single ACT op per k-chunk.
  - Softmax runs without max-subtraction (scores ~ N(0,1); exp of the causal
    -1e9 mask underflows to 0, which is exactly right).
  - Denominators come from a chain with an ALL-ONES [128,128] stationary:
    the row-sum lands replicated across all 128 partitions, so a DVE
    reciprocal_approx_fast + tensor_mul normalizes po with NO transposes.
    The output is written as out[d, q]; the host transposes.
  - Causal variant: fully-masked k-chunks are skipped; the diagonal chunk j
    of a q-block only computes live columns [128j:512] (restricted moving
    operand), and the mask add collapses to one shared [128,128] triangle.
"""

import sys

if "/opt/trn_rl_repo" not in sys.path:
    sys.path.insert(0, "/opt/trn_rl_repo")

import math
import numpy as np

B, S, HID = 2, 2048, 2048
NH, NKV, HD = 16, 4, 128
NCORES = 8
HPC = 4                     # q heads per core (all sharing one kv head)
FPC = HPC * HD              # output features per core
KVW = HD                    # kv head width per core
P = 128
NCH = HID // P              # hid chunks (contraction tiles)
TT = 512                    # token tile, phase 1
QT = 512                    # q tile, phase 2
NKC = S // P                # k chunks

_CACHE = {}


def _build_nc(variant):
    import concourse.bacc as bacc
    from concourse import mybir
    from concourse.tile import TileContext

    f32 = mybir.dt.float32
    bf16 = mybir.dt.bfloat16
    Exp = mybir.ActivationFunctionType.Exp

    nc = bacc.Bacc("TRN2", target_bir_lowering=False, debug=False, num_devices=NCORES)
    xT = nc.dram_tensor("xT", [HID, S], bf16, kind="ExternalInput").ap()
    wq = nc.dram_tensor("wq", [HID, FPC], bf16, kind="ExternalInput").ap()
    wk = nc.dram_tensor("wk", [HID, KVW], bf16, kind="ExternalInput").ap()
    wv = nc.dram_tensor("wv", [HID, KVW], bf16, kind="ExternalInput").ap()
    ident_d = nc.dram_tensor("ident", [P, P], bf16, kind="ExternalInput").ap()
    ones_d = nc.dram_tensor("ones", [P, P], bf16, kind="ExternalInput").ap()
    if variant == "causal":
        tri_d = nc.dram_tensor("tri", [P, P], bf16, kind="ExternalInput").ap()
    if variant == "general":
        maskT = nc.dram_tensor("maskT", [S, S], bf16, kind="ExternalInput").ap()
    out = nc.dram_tensor("out", [FPC, S], f32, kind="ExternalOutput").ap()

    def chunks(q0):
        # [(kc, live_lo)] — live_lo is the first live column within the
        # q-block for that k-chunk (0 = fully live).
        if variant == "causal":
            full = [(kc, 0) for kc in range(q0 // P)]
            diag = [(q0 // P + j, j * P) for j in range(QT // P)]
            return full + diag
        return [(kc, 0) for kc in range(NKC)]

    def is_diag(kc, q0):
        return variant == "causal" and kc >= q0 // P

    with TileContext(nc) as tc:
        with tc.tile_pool(name="persist", bufs=1) as persist:
            wq_sb = persist.tile([P, NCH, FPC], bf16, tag="wq")
            wk_sb = persist.tile([P, NCH, KVW], bf16, tag="wk")
            wv_sb = persist.tile([P, NCH, KVW], bf16, tag="wv")
            ident = persist.tile([P, P], bf16, tag="ident")
            ones_sb = persist.tile([P, P], bf16, tag="ones")
            if variant == "causal":
                tri_sb = persist.tile([P, P], bf16, tag="tri")
            qT_sb = persist.tile([P, HPC, S], bf16, tag="qT")
            kT_sb = persist.tile([P, S], bf16, tag="kT")
            v_sb = persist.tile([P, S], bf16, tag="v")

            nc.gpsimd.dma_start(out=ident[:], in_=ident_d[:])
            nc.gpsimd.dma_start(out=ones_sb[:], in_=ones_d[:])
            if variant == "causal":
                nc.gpsimd.dma_start(out=tri_sb[:], in_=tri_d[:])
            nc.sync.dma_start(out=wq_sb[:], in_=wq.rearrange("(c p) f -> p c f", p=P))
            nc.gpsimd.dma_start(out=wk_sb[:], in_=wk.rearrange("(c p) f -> p c f", p=P))
            nc.gpsimd.dma_start(out=wv_sb[:], in_=wv.rearrange("(c p) f -> p c f", p=P))

            # ---------------- phase 1: Q/K/V projections ----------------
            with tc.tile_pool(name="xt", bufs=2) as xpool, \
                 tc.tile_pool(name="warm", bufs=1, space="PSUM") as wpsum, \
                 tc.tile_pool(name="ppsum", bufs=3, space="PSUM") as ppsum, \
                 tc.tile_pool(name="tpsum", bufs=1, space="PSUM") as tpsum:
                # HAM warmup: harmless matmuls on the identity while the
                # first xT tile's DMA is in flight.
                wp = wpsum.tile([P, P], f32, tag="warm")
                for _ in range(48):
                    nc.tensor.matmul(wp[:], lhsT=ident[:], rhs=ident[:],
                                     start=True, stop=True)
                XSUB = 4                      # hid chunks per xt sub-tile
                NSUB = NCH // XSUB
                for t0 in range(0, S, TT):
                    xts = []
                    for s in range(NSUB):
                        xs = xpool.tile([P, XSUB, TT], bf16, tag=f"xt{s}",
                                        name=f"xt{s}_{t0}")
                        eng = nc.sync if s % 2 == 0 else nc.gpsimd
                        eng.dma_start(
                            out=xs[:],
                            in_=xT[s * XSUB * P:(s + 1) * XSUB * P, t0:t0 + TT]
                            .rearrange("(c p) t -> p c t", p=P),
                        )
                        xts.append(xs)
                    for hp in range(HPC // 2):   # head pairs -> one ACT copy
                        ps = ppsum.tile([P, 2, TT], f32, tag="pp")
                        for h2 in range(2):
                            h = 2 * hp + h2
                            for c in range(NCH):
                                nc.tensor.matmul(
                                    ps[:, h2, :],
                                    lhsT=wq_sb[:, c, h * HD:(h + 1) * HD],
                                    rhs=xts[c // XSUB][:, c % XSUB, :],
                                    start=(c == 0), stop=(c == NCH - 1),
                                )
                        nc.scalar.mul(
                            out=qT_sb[:, 2 * hp:2 * hp + 2, t0:t0 + TT],
                            in_=ps[:], mul=1.0,
                        )
                    ps = ppsum.tile([P, 2, TT], f32, tag="pp")
                    for c in range(NCH):
                        nc.tensor.matmul(
                            ps[:, 0, :], lhsT=wk_sb[:, c, :],
                            rhs=xts[c // XSUB][:, c % XSUB, :],
                            start=(c == 0), stop=(c == NCH - 1),
                        )
                    for c in range(NCH):
                        nc.tensor.matmul(
                            ps[:, 1, :], lhsT=wv_sb[:, c, :],
                            rhs=xts[c // XSUB][:, c % XSUB, :],
                            start=(c == 0), stop=(c == NCH - 1),
                        )
                    nc.scalar.mul(out=kT_sb[:, t0:t0 + TT], in_=ps[:, 0, :], mul=1.0)
                    vt = xpool.tile([P, TT], bf16, tag="vt")
                    nc.vector.tensor_copy(vt[:], ps[:, 1, :])
                    tp = tpsum.tile([P, TT // P, P], bf16, tag="tp")
                    for j in range(TT // P):
                        nc.tensor.transpose(tp[:, j, :], vt[:, j * P:(j + 1) * P], ident[:])
                    nc.vector.tensor_copy(v_sb[:, t0:t0 + TT], tp[:])

            # ---------------- phase 2+3: attention ----------------
            with tc.tile_pool(name="mask", bufs=4) as mpool, \
                 tc.tile_pool(name="et", bufs=1) as epool, \
                 tc.tile_pool(name="rc", bufs=4) as rcpool, \
                 tc.tile_pool(name="ob", bufs=4) as obpool, \
                 tc.tile_pool(name="spsum", bufs=2, space="PSUM") as spsum, \
                 tc.tile_pool(name="opsum", bufs=2, space="PSUM") as opsum, \
                 tc.tile_pool(name="supsum", bufs=2, space="PSUM") as supsum:
                for q0 in range(0, S, QT):
                    act = chunks(q0)
                    et = epool.tile([P, NKC, HPC, QT], bf16, tag="et",
                                    name=f"et_{q0}")
                    # scores + exp, per k-chunk; head pairs share one
                    # 2-bank psum tile so exp is a single ACT op.
                    for kc, lo in act:
                        if variant == "general":
                            mt = mpool.tile([P, QT], bf16, tag="mt")
                            nc.sync.dma_start(
                                out=mt[:], in_=maskT[kc * P:(kc + 1) * P, q0:q0 + QT]
                            )
                        for hp in range(HPC // 2):
                            sp = spsum.tile([P, 2, QT], f32, tag="sp")
                            for h2 in range(2):
                                h = 2 * hp + h2
                                nc.tensor.matmul(
                                    sp[:, h2, lo:],
                                    lhsT=kT_sb[:, kc * P:(kc + 1) * P],
                                    rhs=qT_sb[:, h, q0 + lo:q0 + QT],
                                    start=True, stop=True,
                                )
                                if is_diag(kc, q0):
                                    nc.vector.tensor_add(
                                        out=sp[:, h2, lo:lo + P],
                                        in0=sp[:, h2, lo:lo + P], in1=tri_sb[:],
                                    )
                                elif variant == "general":
                                    nc.vector.tensor_add(
                                        out=sp[:, h2, :], in0=sp[:, h2, :], in1=mt[:]
                                    )
                            nc.scalar.activation(
                                out=et[:, kc, 2 * hp:2 * hp + 2, lo:],
                                in_=sp[:, :, lo:], func=Exp,
                            )
                    # denominators first (reciprocal overlaps attn@V),
                    # then attn @ V (out^T form), normalize, store.
                    for h in range(HPC):
                        sm = supsum.tile([P, QT], f32, tag="sm", name=f"sm{h}_{q0}")
                        for i, (kc, lo) in enumerate(act):
                            nc.tensor.matmul(
                                sm[:, lo:],
                                lhsT=ones_sb[:],
                                rhs=et[:, kc, h, lo:],
                                start=(i == 0), stop=(i == len(act) - 1),
                            )
                        rc = rcpool.tile([P, QT], f32, tag="rc", name=f"rc{h}_{q0}")
                        nc.vector.reciprocal_approx_fast(rc[:], sm[:])
                        po = opsum.tile([P, QT], f32, tag="po", name=f"po{h}_{q0}")
                        for i, (kc, lo) in enumerate(act):
                            nc.tensor.matmul(
                                po[:, lo:],
                                lhsT=v_sb[:, kc * HD:(kc + 1) * HD],
                                rhs=et[:, kc, h, lo:],
                                start=(i == 0), stop=(i == len(act) - 1),
                            )
                        ob = obpool.tile([P, QT], f32, tag="ob", name=f"ob{h}_{q0}")
                        nc.vector.tensor_mul(ob[:], po[:], rc[:])
                        nc.sync.dma_start(
                            out=out[h * HD:(h + 1) * HD, q0:q0 + QT], in_=ob[:]
                        )

    nc.compile()
    return nc


def get_nc(variant="general"):
    if variant not in _CACHE:
        _CACHE[variant] = _build_nc(variant)
    return _CACHE[variant]


def detect_variant(attention_mask):
    m = np.asarray(attention_mask, dtype=np.float32)[:, 0]   # [B, S, S] (q, k)
    if not np.any(m):
        return "zeros"
    # causal: zero on/below the diagonal, <= -1e8 strictly above
    kk = np.arange(S)
    lower = kk[None, :] <= kk[:, None]                       # [S(q), S(k)]
    for b in range(m.shape[0]):
        if np.any(m[b][lower] != 0.0):
            return "general"
        if np.any(m[b][~lower] > -1e8):
            return "general"
    return "causal"


def make_in_maps(variant, hidden_states, attention_mask, Wq, Wk, Wv):
    import ml_dtypes

    bf = ml_dtypes.bfloat16
    xT = np.ascontiguousarray(
        np.asarray(hidden_states, dtype=np.float32).transpose(0, 2, 1)
    ).astype(bf)                                             # [B, HID, S]
    wq_s = (np.asarray(Wq, dtype=np.float32) / math.sqrt(HD)).astype(bf)
    wk = np.asarray(Wk, dtype=np.float32).astype(bf)
    wv = np.asarray(Wv, dtype=np.float32).astype(bf)
    ident = np.eye(P, dtype=np.float32).astype(bf)
    ones = np.ones((P, P), dtype=np.float32).astype(bf)
    rr, cc = np.arange(P)[:, None], np.arange(P)[None, :]
    tri = np.where(rr <= cc, 0.0, -1e9).astype(np.float32).astype(bf)
    if variant == "general":
        mT = np.ascontiguousarray(
            np.asarray(attention_mask, dtype=np.float32)[:, 0].transpose(0, 2, 1)
        ).astype(bf)                                         # [B, S, S]

    in_maps = []
    for c in range(NCORES):
        b, j = c // HPC, c % HPC
        m = {
            "xT": xT[b],
            "wq": np.ascontiguousarray(wq_s[:, j * FPC:(j + 1) * FPC]),
            "wk": np.ascontiguousarray(wk[:, j * KVW:(j + 1) * KVW]),
            "wv": np.ascontiguousarray(wv[:, j * KVW:(j + 1) * KVW]),
            "ident": ident,
            "ones": ones,
        }
        if variant == "causal":
            m["tri"] = tri
        if variant == "general":
            m["maskT"] = mT[b]
        in_maps.append(m)
    return in_maps


def assemble(outs):
    # outs[c] is core c's [FPC, S] slice of out^T for batch c//4.
    per_b = [np.concatenate(outs[b * HPC:(b + 1) * HPC], axis=0) for b in range(B)]
    full = np.stack(per_b, axis=0)                           # [B, HID, S]
    return np.ascontiguousarray(full.transpose(0, 2, 1)).astype(np.float32)


def kernel(hidden_states, attention_mask, Wq, Wk, Wv):
    from concourse.bass_utils import run_bass_kernel_spmd

    variant = detect_variant(attention_mask)
    nc = get_nc(variant)
    in_maps = make_in_maps(variant, hidden_states, attention_mask, Wq, Wk, Wv)
    res = run_bass_kernel_spmd(nc, in_maps, core_ids=list(range(NCORES)))
    return assemble([res.results[c]["out"] for c in range(NCORES)])


# revision 14
# speedup vs baseline: 1.3320x; 1.0156x over previous
"""Trainium2 Bass kernel for CheemsNonWoAttention (GQA attention, no output proj).

Sharding (v4): core c handles batch c//4 and kv-head j=c%4 with its 4 q-heads
4j..4j+3.  KV work is perfectly sharded (no duplication), each core reads only
its batch's activations, SPMD with no collectives.  Each core writes its
[512, S] slice of out^T; the host concatenates and transposes.

Math notes:
  - The reference's logn scale is max(log(65..80)/log(256), 1) == 1.0 -> no-op.
  - 1/sqrt(HD) score scale is folded into Wq on the host.
  - All matmul operands are bf16 (halves HBM traffic and SBUF; PE rate is
    identical to fp32r).  PSUM accumulation stays fp32.
  - Scores are computed transposed, sT[k, q]; exp(sT) feeds attn@V directly
    as the moving operand (V chunks stationary).  Head pairs share one
    2-bank psum tile so exp is a single ACT op per k-chunk.
  - Softmax runs without max-subtraction (scores ~ N(0,1); exp of the causal
    -1e9 mask underflows to 0, which is exactly right).
  - Denominators come from a chain with an ALL-ONES [128,128] stationary:
    the row-sum lands replicated across all 128 partitions, so a DVE
    reciprocal_approx_fast + tensor_mul normalizes po with NO transposes.
    The output is written as out[d, q]; the host transposes.
  - Causal variant: fully-masked k-chunks are skipped; the diagonal chunk j
    of a q-block only computes live columns [128j:512] (restricted moving
    operand), and the mask add collapses to one shared [128,128] triangle.
"""

import sys

if "/opt/trn_rl_repo" not in sys.path:
    sys.path.insert(0, "/opt/trn_rl_repo")

import math
import numpy as np

B, S, HID = 2, 2048, 2048
NH, NKV, HD = 16, 4, 128
NCORES = 8
HPC = 4                     # q heads per core (all sharing one kv head)
FPC = HPC * HD              # output features per core
KVW = HD                    # kv head width per core
P = 128
NCH = HID // P              # hid chunks (contraction tiles)
TT = 512                    # token tile, phase 1
QT = 512                    # q tile, phase 2
NKC = S // P                # k chunks

_CACHE = {}


def _build_nc(variant):
    import concourse.bacc as bacc
    from concourse import mybir
    from concourse.tile import TileContext

    f32 = mybir.dt.float32
    bf16 = mybir.dt.bfloat16
    Exp = mybir.ActivationFunctionType.Exp

    nc = bacc.Bacc("TRN2", target_bir_lowering=False, debug=False, num_devices=NCORES)
    xT = nc.dram_tensor("xT", [HID, S], bf16, kind="ExternalInput").ap()
    wq = nc.dram_tensor("wq", [HID, FPC], bf16, kind="ExternalInput").ap()
    wk = nc.dram_tensor("wk", [HID, KVW], bf16, kind="ExternalInput").ap()
    wv = nc.dram_tensor("wv", [HID, KVW], bf16, kind="ExternalInput").ap()
    ident_d = nc.dram_tensor("ident", [P, P], bf16, kind="ExternalInput").ap()
    ones_d = nc.dram_tensor("ones", [P, P], bf16, kind="ExternalInput").ap()
    if variant == "causal":
        tri_d = nc.dram_tensor("tri", [P, P], bf16, kind="ExternalInput").ap()
    if variant == "general":
        maskT = nc.dram_tensor("maskT", [S, S], bf16, kind="ExternalInput").ap()
    out = nc.dram_tensor("out", [FPC, S], f32, kind="ExternalOutput").ap()

    def chunks(q0):
        # [(kc, live_lo)] — live_lo is the first live column within the
        # q-block for that k-chunk (0 = fully live).
        if variant == "causal":
            full = [(kc, 0) for kc in range(q0 // P)]
            diag = [(q0 // P + j, j * P) for j in range(QT // P)]
            return full + diag
        return [(kc, 0) for kc in range(NKC)]

    def is_diag(kc, q0):
        return variant == "causal" and kc >= q0 // P

    with TileContext(nc) as tc:
        with tc.tile_pool(name="persist", bufs=1) as persist:
            wq_sb = persist.tile([P, NCH, FPC], bf16, tag="wq")
            wk_sb = persist.tile([P, NCH, KVW], bf16, tag="wk")
            wv_sb = persist.tile([P, NCH, KVW], bf16, tag="wv")
            ident = persist.tile([P, P], bf16, tag="ident")
            ones_sb = persist.tile([P, P], bf16, tag="ones")
            if variant == "causal":
                tri_sb = persist.tile([P, P], bf16, tag="tri")
            qT_sb = persist.tile([P, HPC, S], bf16, tag="qT")
            kT_sb = persist.tile([P, S], bf16, tag="kT")
            v_sb = persist.tile([P, S], bf16, tag="v")

            # Queue layout: consts + wk/wv on vector (small, arrive first),
            # xT tiles split sync/gpsimd, wq alone on scalar.  The first K/V
            # chains only need the small weights, so PE starts early.
            nc.gpsimd.dma_start(out=ident[:], in_=ident_d[:])
            nc.gpsimd.dma_start(out=ones_sb[:], in_=ones_d[:])
            if variant == "causal":
                nc.gpsimd.dma_start(out=tri_sb[:], in_=tri_d[:])
            nc.gpsimd.dma_start(out=wk_sb[:], in_=wk.rearrange("(c p) f -> p c f", p=P))
            nc.gpsimd.dma_start(out=wv_sb[:], in_=wv.rearrange("(c p) f -> p c f", p=P))
            nc.scalar.dma_start(out=wq_sb[:], in_=wq.rearrange("(c p) f -> p c f", p=P))

            # ---------------- phase 1: Q/K/V projections ----------------
            with tc.tile_pool(name="xt", bufs=2) as xpool, \
                 tc.tile_pool(name="warm", bufs=1, space="PSUM") as wpsum, \
                 tc.tile_pool(name="ppsum", bufs=3, space="PSUM") as ppsum, \
                 tc.tile_pool(name="tpsum", bufs=1, space="PSUM") as tpsum:
                # HAM warmup: harmless matmuls on the identity while the
                # first xT tile's DMA is in flight.
                wp = wpsum.tile([P, P], f32, tag="warm")
                for _ in range(48):
                    nc.tensor.matmul(wp[:], lhsT=ident[:], rhs=ident[:],
                                     start=True, stop=True)
                XSUB = 4                      # hid chunks per xt sub-tile
                NSUB = NCH // XSUB
                for t0 in range(0, S, TT):
                    xts = []
                    for s in range(NSUB):
                        xs = xpool.tile([P, XSUB, TT], bf16, tag=f"xt{s}",
                                        name=f"xt{s}_{t0}")
                        eng = nc.sync if s % 2 == 0 else nc.gpsimd
                        eng.dma_start(
                            out=xs[:],
                            in_=xT[s * XSUB * P:(s + 1) * XSUB * P, t0:t0 + TT]
                            .rearrange("(c p) t -> p c t", p=P),
                        )
                        xts.append(xs)
                    # K/V first: they only need the small weights, so the
                    # PE can start while wq's DMA is still in flight.
                    ps = ppsum.tile([P, 2, TT], f32, tag="pp")
                    for c in range(NCH):
                        nc.tensor.matmul(
                            ps[:, 0, :], lhsT=wk_sb[:, c, :],
                            rhs=xts[c // XSUB][:, c % XSUB, :],
                            start=(c == 0), stop=(c == NCH - 1),
                        )
                    for c in range(NCH):
                        nc.tensor.matmul(
                            ps[:, 1, :], lhsT=wv_sb[:, c, :],
                            rhs=xts[c // XSUB][:, c % XSUB, :],
                            start=(c == 0), stop=(c == NCH - 1),
                        )
                    nc.scalar.mul(out=kT_sb[:, t0:t0 + TT], in_=ps[:, 0, :], mul=1.0)
                    vt = xpool.tile([P, TT], bf16, tag="vt")
                    nc.vector.tensor_copy(vt[:], ps[:, 1, :])
                    tp = tpsum.tile([P, TT // P, P], bf16, tag="tp")
                    for j in range(TT // P):
                        nc.tensor.transpose(tp[:, j, :], vt[:, j * P:(j + 1) * P], ident[:])
                    nc.vector.tensor_copy(v_sb[:, t0:t0 + TT], tp[:])
                    for hp in range(HPC // 2):   # head pairs -> one ACT copy
                        ps = ppsum.tile([P, 2, TT], f32, tag="pp")
                        for h2 in range(2):
                            h = 2 * hp + h2
                            for c in range(NCH):
                                nc.tensor.matmul(
                                    ps[:, h2, :],
                                    lhsT=wq_sb[:, c, h * HD:(h + 1) * HD],
                                    rhs=xts[c // XSUB][:, c % XSUB, :],
                                    start=(c == 0), stop=(c == NCH - 1),
                                )
                        nc.scalar.mul(
                            out=qT_sb[:, 2 * hp:2 * hp + 2, t0:t0 + TT],
                            in_=ps[:], mul=1.0,
                        )

            # ---------------- phase 2+3: attention ----------------
            # Two head-pair passes per q-block, software-pipelined per
            # k-chunk: the sums/attn@V chain steps for chunk i-LAG are
            # emitted between the scores+exp of chunk i, so the PE never
            # waits on the ACT engine's exp.  et is chunk-granular so
            # the dependencies stay chunk-level.
            LAG = 2
            with tc.tile_pool(name="mask", bufs=4) as mpool, \
                 tc.tile_pool(name="et", bufs=2) as epool, \
                 tc.tile_pool(name="rc", bufs=4) as rcpool, \
                 tc.tile_pool(name="ob", bufs=4) as obpool, \
                 tc.tile_pool(name="spsum", bufs=2, space="PSUM") as spsum, \
                 tc.tile_pool(name="opsum", bufs=1, space="PSUM") as opsum, \
                 tc.tile_pool(name="supsum", bufs=1, space="PSUM") as supsum:
                for q0 in range(0, S, QT):
                    act = chunks(q0)
                    n = len(act)
                    masks = {}
                    if variant == "general":
                        for kc, _ in act:
                            mt = mpool.tile([P, QT], bf16, tag=f"mt{kc}",
                                            name=f"mt{kc}_{q0}")
                            nc.sync.dma_start(
                                out=mt[:],
                                in_=maskT[kc * P:(kc + 1) * P, q0:q0 + QT],
                            )
                            masks[kc] = mt
                    for hp in range(HPC // 2):
                        h0, h1 = 2 * hp, 2 * hp + 1
                        ets = {}
                        sm = {h: supsum.tile([P, QT], f32, tag=f"sm{h % 2}",
                                             name=f"sm{h}_{q0}")
                              for h in (h0, h1)}
                        po = {h: opsum.tile([P, QT], f32, tag=f"po{h % 2}",
                                            name=f"po{h}_{q0}")
                              for h in (h0, h1)}
                        for i in range(n + LAG):
                            if i < n:
                                kc, lo = act[i]
                                sp = spsum.tile([P, 2, QT], f32, tag="sp")
                                e = epool.tile([P, 2, QT], bf16, tag=f"et{kc % 4}",
                                               name=f"et{kc}_{hp}_{q0}")
                                ets[kc] = e
                                for h2 in range(2):
                                    nc.tensor.matmul(
                                        sp[:, h2, lo:],
                                        lhsT=kT_sb[:, kc * P:(kc + 1) * P],
                                        rhs=qT_sb[:, 2 * hp + h2, q0 + lo:q0 + QT],
                                        start=True, stop=True,
                                    )
                                    if is_diag(kc, q0):
                                        nc.vector.tensor_add(
                                            out=sp[:, h2, lo:lo + P],
                                            in0=sp[:, h2, lo:lo + P], in1=tri_sb[:],
                                        )
                                    elif variant == "general":
                                        nc.vector.tensor_add(
                                            out=sp[:, h2, :], in0=sp[:, h2, :],
                                            in1=masks[kc][:],
                                        )
                                nc.scalar.activation(
                                    out=e[:, :, lo:], in_=sp[:, :, lo:], func=Exp
                                )
                            j = i - LAG
                            if 0 <= j < n:
                                kc, lo = act[j]
                                e = ets[kc]
                                for h2 in range(2):
                                    nc.tensor.matmul(
                                        sm[2 * hp + h2][:, lo:],
                                        lhsT=ones_sb[:],
                                        rhs=e[:, h2, lo:],
                                        start=(j == 0), stop=(j == n - 1),
                                    )
                                for h2 in range(2):
                                    nc.tensor.matmul(
                                        po[2 * hp + h2][:, lo:],
                                        lhsT=v_sb[:, kc * HD:(kc + 1) * HD],
                                        rhs=e[:, h2, lo:],
                                        start=(j == 0), stop=(j == n - 1),
                                    )
                        for h in (h0, h1):
                            rc = rcpool.tile([P, QT], f32, tag="rc",
                                             name=f"rc{h}_{q0}")
                            nc.vector.reciprocal_approx_fast(rc[:], sm[h][:])
                            ob = obpool.tile([P, QT], f32, tag="ob",
                                             name=f"ob{h}_{q0}")
                            nc.vector.tensor_mul(ob[:], po[h][:], rc[:])
                            nc.sync.dma_start(
                                out=out[h * HD:(h + 1) * HD, q0:q0 + QT], in_=ob[:]
                            )

    nc.compile()
    return nc


def get_nc(variant="general"):
    if variant not in _CACHE:
        _CACHE[variant] = _build_nc(variant)
    return _CACHE[variant]


def detect_variant(attention_mask):
    m = np.asarray(attention_mask, dtype=np.float32)[:, 0]   # [B, S, S] (q, k)
    if not np.any(m):
        return "zeros"
    # causal: zero on/below the diagonal, <= -1e8 strictly above
    kk = np.arange(S)
    lower = kk[None, :] <= kk[:, None]                       # [S(q), S(k)]
    for b in range(m.shape[0]):
        if np.any(m[b][lower] != 0.0):
            return "general"
        if np.any(m[b][~lower] > -1e8):
            return "general"
    return "causal"


def make_in_maps(variant, hidden_states, attention_mask, Wq, Wk, Wv):
    import ml_dtypes

    bf = ml_dtypes.bfloat16
    xT = np.ascontiguousarray(
        np.asarray(hidden_states, dtype=np.float32).transpose(0, 2, 1)
    ).astype(bf)                                             # [B, HID, S]
    wq_s = (np.asarray(Wq, dtype=np.float32) / math.sqrt(HD)).astype(bf)
    wk = np.asarray(Wk, dtype=np.float32).astype(bf)
    wv = np.asarray(Wv, dtype=np.float32).astype(bf)
    ident = np.eye(P, dtype=np.float32).astype(bf)
    ones = np.ones((P, P), dtype=np.float32).astype(bf)
    rr, cc = np.arange(P)[:, None], np.arange(P)[None, :]
    tri = np.where(rr <= cc, 0.0, -1e9).astype(np.float32).astype(bf)
    if variant == "general":
        mT = np.ascontiguousarray(
            np.asarray(attention_mask, dtype=np.float32)[:, 0].transpose(0, 2, 1)
        ).astype(bf)                                         # [B, S, S]

    in_maps = []
    for c in range(NCORES):
        b, j = c // HPC, c % HPC
        m = {
            "xT": xT[b],
            "wq": np.ascontiguousarray(wq_s[:, j * FPC:(j + 1) * FPC]),
            "wk": np.ascontiguousarray(wk[:, j * KVW:(j + 1) * KVW]),
            "wv": np.ascontiguousarray(wv[:, j * KVW:(j + 1) * KVW]),
            "ident": ident,
            "ones": ones,
        }
        if variant == "causal":
            m["tri"] = tri
        if variant == "general":
            m["maskT"] = mT[b]
        in_maps.append(m)
    return in_maps


def assemble(outs):
    # outs[c] is core c's [FPC, S] slice of out^T for batch c//4.
    per_b = [np.concatenate(outs[b * HPC:(b + 1) * HPC], axis=0) for b in range(B)]
    full = np.stack(per_b, axis=0)                           # [B, HID, S]
    return np.ascontiguousarray(full.transpose(0, 2, 1)).astype(np.float32)


def kernel(hidden_states, attention_mask, Wq, Wk, Wv):
    from concourse.bass_utils import run_bass_kernel_spmd

    variant = detect_variant(attention_mask)
    nc = get_nc(variant)
    in_maps = make_in_maps(variant, hidden_states, attention_mask, Wq, Wk, Wv)
    res = run_bass_kernel_spmd(nc, in_maps, core_ids=list(range(NCORES)))
    return assemble([res.results[c]["out"] for c in range(NCORES)])


# revision 21
# speedup vs baseline: 1.4400x; 1.0810x over previous
"""Trainium2 Bass kernel for CheemsNonWoAttention (GQA attention, no output proj).

Sharding (v4): core c handles batch c//4 and kv-head j=c%4 with its 4 q-heads
4j..4j+3.  KV work is perfectly sharded (no duplication), each core reads only
its batch's activations, SPMD with no collectives.  Each core writes its
[512, S] slice of out^T; the host concatenates and transposes.

Math notes:
  - The reference's logn scale is max(log(65..80)/log(256), 1) == 1.0 -> no-op.
  - 1/sqrt(HD) score scale is folded into Wq on the host.
  - All matmul operands are bf16 (halves HBM traffic and SBUF; PE rate is
    identical to fp32r).  PSUM accumulation stays fp32.
  - Scores are computed transposed, sT[k, q]; exp(sT) feeds attn@V directly
    as the moving operand (V chunks stationary).  Head pairs share one
    2-bank psum tile so exp is a single ACT op per k-chunk.
  - Softmax runs without max-subtraction (scores ~ N(0,1); exp of the causal
    -1e9 mask underflows to 0, which is exactly right).
  - Denominators come from a chain with an ALL-ONES [128,128] stationary:
    the row-sum lands replicated across all 128 partitions, so a DVE
    reciprocal_approx_fast + tensor_mul normalizes po with NO transposes.
    The output is written as out[d, q]; the host transposes.
  - Causal variant: fully-masked k-chunks are skipped; the diagonal chunk j
    of a q-block only computes live columns [128j:512] (restricted moving
    operand), and the mask add collapses to one shared [128,128] triangle.
"""

import sys

if "/opt/trn_rl_repo" not in sys.path:
    sys.path.insert(0, "/opt/trn_rl_repo")

import math
import numpy as np

B, S, HID = 2, 2048, 2048
NH, NKV, HD = 16, 4, 128
NCORES = 8
HPC = 4                     # q heads per core (all sharing one kv head)
FPC = HPC * HD              # output features per core
KVW = HD                    # kv head width per core
P = 128
NCH = HID // P              # hid chunks (contraction tiles)
TT = 512                    # token tile, phase 1
QT = 512                    # q tile, phase 2
NKC = S // P                # k chunks

_CACHE = {}


def _build_nc(variant):
    import concourse.bacc as bacc
    from concourse import mybir
    from concourse.tile import TileContext

    f32 = mybir.dt.float32
    bf16 = mybir.dt.bfloat16
    Exp = mybir.ActivationFunctionType.Exp

    nc = bacc.Bacc("TRN2", target_bir_lowering=False, debug=False, num_devices=NCORES)
    xT = nc.dram_tensor("xT", [HID, S], bf16, kind="ExternalInput").ap()
    # weights come pre-chunked from the host: [P, ...] with contiguous
    # per-partition lines so the DMA moves 4-16KB elements, not 256B.
    wq = nc.dram_tensor("wq", [P, 2, NCH, FPC // 2], bf16, kind="ExternalInput").ap()
    wk = nc.dram_tensor("wk", [P, NCH, KVW], bf16, kind="ExternalInput").ap()
    wv = nc.dram_tensor("wv", [P, NCH, KVW], bf16, kind="ExternalInput").ap()
    ident_d = nc.dram_tensor("ident", [P, P], bf16, kind="ExternalInput").ap()
    ones_d = nc.dram_tensor("ones", [P, P], bf16, kind="ExternalInput").ap()
    if variant == "causal":
        tri_d = nc.dram_tensor("tri", [P, P], bf16, kind="ExternalInput").ap()
    if variant == "general":
        maskT = nc.dram_tensor("maskT", [S, S], bf16, kind="ExternalInput").ap()
    out = nc.dram_tensor("out", [FPC, S], f32, kind="ExternalOutput").ap()

    def chunks(q0):
        # [(kc, live_lo)] — live_lo is the first live column within the
        # q-block for that k-chunk (0 = fully live).
        if variant == "causal":
            full = [(kc, 0) for kc in range(q0 // P)]
            diag = [(q0 // P + j, j * P) for j in range(QT // P)]
            return full + diag
        return [(kc, 0) for kc in range(NKC)]

    def is_diag(kc, q0):
        return variant == "causal" and kc >= q0 // P

    with TileContext(nc) as tc:
        with tc.tile_pool(name="persist", bufs=1) as persist:
            wq_sb = persist.tile([P, 2, NCH, FPC // 2], bf16, tag="wq")
            wk_sb = persist.tile([P, NCH, KVW], bf16, tag="wk")
            wv_sb = persist.tile([P, NCH, KVW], bf16, tag="wv")
            ident = persist.tile([P, P], bf16, tag="ident")
            ones_sb = persist.tile([P, P], bf16, tag="ones")
            if variant == "causal":
                tri_sb = persist.tile([P, P], bf16, tag="tri")
            qT_sb = persist.tile([P, HPC, S], bf16, tag="qT")
            kT_sb = persist.tile([P, S], bf16, tag="kT")
            v_sb = persist.tile([P, S], bf16, tag="v")

            # Everything bulk goes on the sync hardware-DGE queue (it
            # stripes across DMA engines at ~270GB/s; splitting engines
            # measured slower).  Issue order is arrival order: small
            # K/V weights, first xT tile, then wq per head-pair (each Q
            # chain starts as its half lands), then later xT tiles.
            nc.gpsimd.dma_start(out=ident[:], in_=ident_d[:])
            nc.gpsimd.dma_start(out=ones_sb[:], in_=ones_d[:])
            if variant == "causal":
                nc.gpsimd.dma_start(out=tri_sb[:], in_=tri_d[:])
            nc.sync.dma_start(out=wk_sb[:], in_=wk[:])
            nc.sync.dma_start(out=wv_sb[:], in_=wv[:])

            # ---------------- phase 1: Q/K/V projections ----------------
            with tc.tile_pool(name="xt", bufs=2) as xpool, \
                 tc.tile_pool(name="warm", bufs=1, space="PSUM") as wpsum, \
                 tc.tile_pool(name="ppsum", bufs=3, space="PSUM") as ppsum, \
                 tc.tile_pool(name="tpsum", bufs=1, space="PSUM") as tpsum:
                # HAM warmup: harmless matmuls on the identity while the
                # first xT tile's DMA is in flight.
                wp = wpsum.tile([P, P], f32, tag="warm")
                for _ in range(64):
                    nc.tensor.matmul(wp[:], lhsT=ident[:], rhs=ident[:],
                                     start=True, stop=True)
                XSUB = 4                      # hid chunks per xt sub-tile
                NSUB = NCH // XSUB
                for t0 in range(0, S, TT):
                    xts = []
                    for s in range(NSUB):
                        xs = xpool.tile([P, XSUB, TT], bf16, tag=f"xt{s}",
                                        name=f"xt{s}_{t0}")
                        nc.sync.dma_start(
                            out=xs[:],
                            in_=xT[s * XSUB * P:(s + 1) * XSUB * P, t0:t0 + TT]
                            .rearrange("(c p) t -> p c t", p=P),
                        )
                        xts.append(xs)
                    if t0 == 0:
                        # wq lands per head-pair while the K/V chains run.
                        for hp in range(2):
                            nc.sync.dma_start(
                                out=wq_sb[:, hp], in_=wq[:, hp]
                            )
                    # K/V first: they only need the small weights, so the
                    # PE can start while wq's DMA is still in flight.
                    ps = ppsum.tile([P, 2, TT], f32, tag="pp")
                    for c in range(NCH):
                        nc.tensor.matmul(
                            ps[:, 0, :], lhsT=wk_sb[:, c, :],
                            rhs=xts[c // XSUB][:, c % XSUB, :],
                            start=(c == 0), stop=(c == NCH - 1),
                        )
                    for c in range(NCH):
                        nc.tensor.matmul(
                            ps[:, 1, :], lhsT=wv_sb[:, c, :],
                            rhs=xts[c // XSUB][:, c % XSUB, :],
                            start=(c == 0), stop=(c == NCH - 1),
                        )
                    nc.scalar.mul(out=kT_sb[:, t0:t0 + TT], in_=ps[:, 0, :], mul=1.0)
                    vt = xpool.tile([P, TT], bf16, tag="vt")
                    nc.vector.tensor_copy(vt[:], ps[:, 1, :])
                    tp = tpsum.tile([P, TT // P, P], bf16, tag="tp")
                    for j in range(TT // P):
                        nc.tensor.transpose(tp[:, j, :], vt[:, j * P:(j + 1) * P], ident[:])
                    nc.vector.tensor_copy(v_sb[:, t0:t0 + TT], tp[:])
                    for hp in range(HPC // 2):   # head pairs -> one ACT copy
                        ps = ppsum.tile([P, 2, TT], f32, tag="pp")
                        for h2 in range(2):
                            h = 2 * hp + h2
                            for c in range(NCH):
                                nc.tensor.matmul(
                                    ps[:, h2, :],
                                    lhsT=wq_sb[:, hp, c, h2 * HD:(h2 + 1) * HD],
                                    rhs=xts[c // XSUB][:, c % XSUB, :],
                                    start=(c == 0), stop=(c == NCH - 1),
                                )
                        nc.scalar.mul(
                            out=qT_sb[:, 2 * hp:2 * hp + 2, t0:t0 + TT],
                            in_=ps[:], mul=1.0,
                        )

            # ---------------- phase 2+3: attention ----------------
            # Two head-pair passes per q-block, software-pipelined per
            # k-chunk: the sums/attn@V chain steps for chunk i-LAG are
            # emitted between the scores+exp of chunk i, so the PE never
            # waits on the ACT engine's exp.  et is chunk-granular so
            # the dependencies stay chunk-level.
            LAG = 2
            with tc.tile_pool(name="mask", bufs=4) as mpool, \
                 tc.tile_pool(name="et", bufs=2) as epool, \
                 tc.tile_pool(name="rc", bufs=4) as rcpool, \
                 tc.tile_pool(name="ob", bufs=4) as obpool, \
                 tc.tile_pool(name="spsum", bufs=2, space="PSUM") as spsum, \
                 tc.tile_pool(name="opsum", bufs=1, space="PSUM") as opsum, \
                 tc.tile_pool(name="supsum", bufs=1, space="PSUM") as supsum:
                for q0 in range(0, S, QT):
                    act = chunks(q0)
                    n = len(act)
                    masks = {}
                    if variant == "general":
                        for kc, _ in act:
                            mt = mpool.tile([P, QT], bf16, tag=f"mt{kc}",
                                            name=f"mt{kc}_{q0}")
                            nc.sync.dma_start(
                                out=mt[:],
                                in_=maskT[kc * P:(kc + 1) * P, q0:q0 + QT],
                            )
                            masks[kc] = mt
                    for hp in range(HPC // 2):
                        h0, h1 = 2 * hp, 2 * hp + 1
                        ets = {}
                        sm = {h: supsum.tile([P, QT], f32, tag=f"sm{h % 2}",
                                             name=f"sm{h}_{q0}")
                              for h in (h0, h1)}
                        po = {h: opsum.tile([P, QT], f32, tag=f"po{h % 2}",
                                            name=f"po{h}_{q0}")
                              for h in (h0, h1)}
                        for i in range(n + LAG):
                            if i < n:
                                kc, lo = act[i]
                                sp = spsum.tile([P, 2, QT], f32, tag="sp")
                                e = epool.tile([P, 2, QT], bf16, tag=f"et{kc % 4}",
                                               name=f"et{kc}_{hp}_{q0}")
                                ets[kc] = e
                                for h2 in range(2):
                                    nc.tensor.matmul(
                                        sp[:, h2, lo:],
                                        lhsT=kT_sb[:, kc * P:(kc + 1) * P],
                                        rhs=qT_sb[:, 2 * hp + h2, q0 + lo:q0 + QT],
                                        start=True, stop=True,
                                    )
                                    if is_diag(kc, q0):
                                        nc.vector.tensor_add(
                                            out=sp[:, h2, lo:lo + P],
                                            in0=sp[:, h2, lo:lo + P], in1=tri_sb[:],
                                        )
                                    elif variant == "general":
                                        nc.vector.tensor_add(
                                            out=sp[:, h2, :], in0=sp[:, h2, :],
                                            in1=masks[kc][:],
                                        )
                                nc.scalar.activation(
                                    out=e[:, :, lo:], in_=sp[:, :, lo:], func=Exp
                                )
                            j = i - LAG
                            if 0 <= j < n:
                                kc, lo = act[j]
                                e = ets[kc]
                                for h2 in range(2):
                                    nc.tensor.matmul(
                                        sm[2 * hp + h2][:, lo:],
                                        lhsT=ones_sb[:],
                                        rhs=e[:, h2, lo:],
                                        start=(j == 0), stop=(j == n - 1),
                                    )
                                for h2 in range(2):
                                    nc.tensor.matmul(
                                        po[2 * hp + h2][:, lo:],
                                        lhsT=v_sb[:, kc * HD:(kc + 1) * HD],
                                        rhs=e[:, h2, lo:],
                                        start=(j == 0), stop=(j == n - 1),
                                    )
                        for h in (h0, h1):
                            rc = rcpool.tile([P, QT], f32, tag="rc",
                                             name=f"rc{h}_{q0}")
                            nc.vector.reciprocal_approx_fast(rc[:], sm[h][:])
                            ob = obpool.tile([P, QT], f32, tag="ob",
                                             name=f"ob{h}_{q0}")
                            nc.vector.tensor_mul(ob[:], po[h][:], rc[:])
                            nc.sync.dma_start(
                                out=out[h * HD:(h + 1) * HD, q0:q0 + QT], in_=ob[:]
                            )

    nc.compile()
    return nc


def get_nc(variant="general"):
    if variant not in _CACHE:
        _CACHE[variant] = _build_nc(variant)
    return _CACHE[variant]


def detect_variant(attention_mask):
    m = np.asarray(attention_mask, dtype=np.float32)[:, 0]   # [B, S, S] (q, k)
    if not np.any(m):
        return "zeros"
    # causal: zero on/below the diagonal, <= -1e8 strictly above
    kk = np.arange(S)
    lower = kk[None, :] <= kk[:, None]                       # [S(q), S(k)]
    for b in range(m.shape[0]):
        if np.any(m[b][lower] != 0.0):
            return "general"
        if np.any(m[b][~lower] > -1e8):
            return "general"
    return "causal"


def make_in_maps(variant, hidden_states, attention_mask, Wq, Wk, Wv):
    import ml_dtypes

    bf = ml_dtypes.bfloat16
    xT = np.ascontiguousarray(
        np.asarray(hidden_states, dtype=np.float32).transpose(0, 2, 1)
    ).astype(bf)                                             # [B, HID, S]
    wq_s = (np.asarray(Wq, dtype=np.float32) / math.sqrt(HD)).astype(bf)
    wk = np.asarray(Wk, dtype=np.float32).astype(bf)
    wv = np.asarray(Wv, dtype=np.float32).astype(bf)
    ident = np.eye(P, dtype=np.float32).astype(bf)
    ones = np.ones((P, P), dtype=np.float32).astype(bf)
    rr, cc = np.arange(P)[:, None], np.arange(P)[None, :]
    tri = np.where(rr <= cc, 0.0, -1e9).astype(np.float32).astype(bf)
    if variant == "general":
        mT = np.ascontiguousarray(
            np.asarray(attention_mask, dtype=np.float32)[:, 0].transpose(0, 2, 1)
        ).astype(bf)                                         # [B, S, S]

    def chunked(w):
        # [HID, F] -> [P, NCH, F] with contiguous per-partition lines
        return np.ascontiguousarray(
            w.reshape(NCH, P, w.shape[1]).transpose(1, 0, 2)
        )

    in_maps = []
    for c in range(NCORES):
        b, j = c // HPC, c % HPC
        wq_c = wq_s[:, j * FPC:(j + 1) * FPC]                # [HID, 512]
        wq_c = np.ascontiguousarray(
            wq_c.reshape(NCH, P, 2, FPC // 2).transpose(1, 2, 0, 3)
        )                                                    # [P, 2, NCH, 256]
        m = {
            "xT": xT[b],
            "wq": wq_c,
            "wk": chunked(wk[:, j * KVW:(j + 1) * KVW]),
            "wv": chunked(wv[:, j * KVW:(j + 1) * KVW]),
            "ident": ident,
            "ones": ones,
        }
        if variant == "causal":
            m["tri"] = tri
        if variant == "general":
            m["maskT"] = mT[b]
        in_maps.append(m)
    return in_maps


def assemble(outs):
    # outs[c] is core c's [FPC, S] slice of out^T for batch c//4.
    per_b = [np.concatenate(outs[b * HPC:(b + 1) * HPC], axis=0) for b in range(B)]
    full = np.stack(per_b, axis=0)                           # [B, HID, S]
    return np.ascontiguousarray(full.transpose(0, 2, 1)).astype(np.float32)


def kernel(hidden_states, attention_mask, Wq, Wk, Wv):
    from concourse.bass_utils import run_bass_kernel_spmd

    variant = detect_variant(attention_mask)
    nc = get_nc(variant)
    in_maps = make_in_maps(variant, hidden_states, attention_mask, Wq, Wk, Wv)
    res = run_bass_kernel_spmd(nc, in_maps, core_ids=list(range(NCORES)))
    return assemble([res.results[c]["out"] for c in range(NCORES)])
